# revision 1
# baseline (speedup 1.0000x reference)
"""Co-teaching loss (drop-region CE) kernel for Trainium2, 8 NeuronCores.

Reference computation:
  - 2x2 maxpool on inputs1/inputs2 [8,19,512,512] and targets [8,512,512]
  - per-pixel CE loss of each pooled input vs pooled targets -> [8, 65536] x2
  - per-row ascending argsort of each loss map, keep num_remember smallest,
    gather the *other* loss at those indices, return the two scalar means.

Distribution: data-parallel over batch B=8, one batch row per NeuronCore.
Each core computes its row's two pooled CE loss maps [256,256] on-device
(that is all of the memory-bound work: ~41MB of input reads per core).
The tiny top-k selection over the [8, 65536] loss maps (0.5 MB/core out)
is done on host exactly like the reference (stable argsort semantics).

On-device pipeline per core (f32):
  - inputs arrive as [128 h-pair partitions, (channel, w)] tiles: even input
    rows in the first half of the tile, odd rows in the second half.
  - h-pool: in-place tensor_tensor max on DVE; w-pool: strided TT max (DVE).
  - targets pooled the same way on GPSIMD (integer TT max is Pool-legal).
  - CE: one-hot masks (tp == c) on GPSIMD; x_t assembled with
    copy_predicated (DVE); logsumexp via in-place Exp (ACT) + tensor_reduce
    over channels (DVE, channel innermost) + Ln (ACT); loss = logS - x_t.
"""

import numpy as np

B, C, H, W = 8, 19, 512, 512
HP, WP = 256, 256  # pooled spatial dims
L = HP * WP
N_CORES = 8
# channel groups per DMA/pool unit (sum = 19)
GROUPS = [(0, 4), (4, 4), (8, 4), (12, 4), (16, 3)]

_prog_cache = {}

USE_BF16 = True  # pool/gather in bf16 (cast during DMA); CE sums in f32


def _build_program(repeat=1):
    from contextlib import ExitStack

    import concourse.bass as bass  # noqa: F401
    import concourse.mybir as mybir
    import concourse.tile as tile
    from concourse import bacc

    f32 = mybir.dt.float32
    i32 = mybir.dt.int32
    u8 = mybir.dt.uint8
    Alu = mybir.AluOpType
    Act = mybir.ActivationFunctionType
    pdt = mybir.dt.bfloat16 if USE_BF16 else f32  # pooling datapath dtype

    nc = bacc.Bacc("TRN2", target_bir_lowering=False, debug=False,
                   num_devices=N_CORES)

    x_in = [
        nc.dram_tensor("x1", [C, H, W], f32, kind="ExternalInput"),
        nc.dram_tensor("x2", [C, H, W], f32, kind="ExternalInput"),
    ]
    tg = nc.dram_tensor("tg", [H, W], i32, kind="ExternalInput")
    l_out = [
        nc.dram_tensor("loss1", [HP, WP], f32, kind="ExternalOutput"),
        nc.dram_tensor("loss2", [HP, WP], f32, kind="ExternalOutput"),
    ]

    CW = C * WP  # 4864: one input's pooled row block inside P12

    with tile.TileContext(nc) as tc, ExitStack() as ctx:
        raw_pool = ctx.enter_context(
            tc.tile_pool(name="raw", bufs=9 if USE_BF16 else 4))
        p_pool = ctx.enter_context(tc.tile_pool(name="pooled", bufs=2))
        tgt_pool = ctx.enter_context(tc.tile_pool(name="tgt", bufs=2))
        mask_pool = ctx.enter_context(tc.tile_pool(name="mask", bufs=2))
        small = ctx.enter_context(tc.tile_pool(name="small", bufs=2))

        for half in [h for _ in range(repeat) for h in range(2)]:
            ib = 256 * half      # first input row of this half
            ob = 128 * half      # first pooled row of this half

            # ---- pooled targets for this half: tp [128, 256] int32 ----
            # contiguous 512KB half-slab -> [128, 1024]; partition p holds
            # rows (ib+2p, ib+2p+1) back to back (4KB contiguous per part)
            trow = tgt_pool.tile([128, 2 * W], i32, tag="traw")
            nc.sync.dma_start(
                out=trow[:],
                in_=tg[ib:ib + 256, :].rearrange("h w -> (h w)")
                .rearrange("(p n) -> p n", p=128))
            nc.vector.tensor_tensor(out=trow[:, :W], in0=trow[:, :W],
                                    in1=trow[:, W:], op=Alu.max)
            tp = tgt_pool.tile([128, WP], i32, tag="tp")
            nc.vector.tensor_tensor(out=tp[:], in0=trow[:, 0:W:2],
                                    in1=trow[:, 1:W:2], op=Alu.max)

            # ---- masks (tp == c) for c = 1..18, shared by both inputs ----
            masks = mask_pool.tile([128, 18 * WP], u8, tag="masks")
            for c in range(1, C):
                nc.gpsimd.tensor_scalar(
                    out=masks[:, (c - 1) * WP:c * WP], in0=tp[:],
                    scalar1=float(c), scalar2=None, op0=Alu.is_equal)

            # ---- pooled logits, both inputs in one tile: P12[p,(xi,c,w)]
            # per-channel contiguous 512KB DMAs into grouped tiles, then
            # one h-pool + one w-pool DVE op per channel group.
            # CE work (x_t gather, exp) is done per-group as data lands so
            # only a tiny tail depends on the last-loaded channels.
            P12 = p_pool.tile([128, 2 * CW], pdt, tag="P12")
            xt12 = small.tile([128, 2 * WP], pdt, tag="xt")
            for xi in range(2):
                xt = xt12[:, xi * WP:(xi + 1) * WP]
                for (c0, G) in GROUPS:
                    T = raw_pool.tile([128, 4 * 2 * W], pdt, tag="T")
                    for ci in range(G):
                        src = x_in[xi][c0 + ci, ib:ib + 256, :] \
                            .rearrange("h w -> (h w)") \
                            .rearrange("(p n) -> p n", p=128)
                        if USE_BF16:  # SWDGE cast DMA f32 -> bf16
                            nc.gpsimd.dma_start(
                                out=T[:, ci * 2 * W:(ci + 1) * 2 * W],
                                in_=src)
                        else:
                            nc.sync.dma_start(
                                out=T[:, ci * 2 * W:(ci + 1) * 2 * W],
                                in_=src)
                    Tv = T[:, :G * 2 * W].rearrange(
                        "p (c n) -> p c n", c=G)
                    # h-pool in place (row-parity max), then strided w-pool
                    nc.vector.tensor_tensor(
                        out=Tv[:, :, 0:W], in0=Tv[:, :, 0:W],
                        in1=Tv[:, :, W:2 * W], op=Alu.max)
                    nc.vector.tensor_tensor(
                        out=P12[:, xi * CW + c0 * WP:xi * CW + (c0 + G) * WP],
                        in0=Tv[:, :, 0:W:2], in1=Tv[:, :, 1:W:2], op=Alu.max)
                    # x_t updates for this group's channels
                    if c0 == 0:
                        nc.vector.tensor_copy(xt,
                                              P12[:, xi * CW:xi * CW + WP])
                    for c in range(max(c0, 1), c0 + G):
                        nc.vector.copy_predicated(
                            out=xt, mask=masks[:, (c - 1) * WP:c * WP],
                            data=P12[:, xi * CW + c * WP:
                                     xi * CW + (c + 1) * WP])
                    # exp this group in place (x_t already extracted)
                    nc.scalar.activation(
                        out=P12[:, xi * CW + c0 * WP:xi * CW + (c0 + G) * WP],
                        in_=P12[:, xi * CW + c0 * WP:xi * CW + (c0 + G) * WP],
                        func=Act.Exp)

            # ---- logsumexp: bulk reduce over c<16 overlaps the last loads,
            # only the 3-channel tail depends on the final group ----
            CB = 16  # bulk channels
            S12p = small.tile([128, 2 * WP], f32, tag="Sp")
            nc.vector.tensor_reduce(
                out=S12p[:],
                in_=P12[:].rearrange("p (x c w) -> p x w c", x=2, c=C)
                [:, :, :, 0:CB],
                axis=mybir.AxisListType.X, op=Alu.add)
            S12t = small.tile([128, 2 * WP], f32, tag="St")
            nc.vector.tensor_reduce(
                out=S12t[:],
                in_=P12[:].rearrange("p (x c w) -> p x w c", x=2, c=C)
                [:, :, :, CB:C],
                axis=mybir.AxisListType.X, op=Alu.add)
            nc.vector.tensor_add(S12p[:], S12p[:], S12t[:])
            logS12 = small.tile([128, 2 * WP], f32, tag="logS")
            nc.scalar.activation(out=logS12[:], in_=S12p[:], func=Act.Ln)
            lt12 = small.tile([128, 2 * WP], f32, tag="loss")
            nc.vector.tensor_sub(lt12[:], logS12[:], xt12[:])
            for xi in range(2):
                nc.sync.dma_start(out=l_out[xi][ob:ob + 128, :],
                                  in_=lt12[:, xi * WP:(xi + 1) * WP])

    nc.compile()
    return nc


def _get_program():
    if "nc" not in _prog_cache:
        _prog_cache["nc"] = _build_program()
    return _prog_cache["nc"]


def _device_loss_maps(inputs1, inputs2, targets):
    """Run the 8-core SPMD kernel; return loss1, loss2 as [8, 65536] f32."""
    from concourse.bass_utils import run_bass_kernel_spmd

    nc = _get_program()
    in_maps = [
        {
            "x1": np.ascontiguousarray(inputs1[b], dtype=np.float32),
            "x2": np.ascontiguousarray(inputs2[b], dtype=np.float32),
            "tg": np.ascontiguousarray(targets[b], dtype=np.int32),
        }
        for b in range(B)
    ]
    res = run_bass_kernel_spmd(nc, in_maps, list(range(N_CORES)))
    loss1 = np.stack([np.asarray(res.results[b]["loss1"]).reshape(L)
                      for b in range(B)])
    loss2 = np.stack([np.asarray(res.results[b]["loss2"]).reshape(L)
                      for b in range(B)])
    return loss1, loss2


def kernel(inputs1, inputs2, targets, forget_rate):
    inputs1 = np.asarray(inputs1, dtype=np.float32)
    inputs2 = np.asarray(inputs2, dtype=np.float32)
    targets = np.asarray(targets, dtype=np.int32)

    loss1, loss2 = _device_loss_maps(inputs1, inputs2, targets)

    num_remember = int((1.0 - float(forget_rate)) * L)
    # stable ascending argsort (matches jnp.argsort) -> keep smallest k,
    # gather the swapped loss, mean.
    ind1 = np.argsort(loss1, axis=1, kind="stable")[:, :num_remember]
    ind2 = np.argsort(loss2, axis=1, kind="stable")[:, :num_remember]
    m1 = np.take_along_axis(loss1, ind2, axis=1).mean(dtype=np.float64)
    m2 = np.take_along_axis(loss2, ind1, axis=1).mean(dtype=np.float64)
    return np.array([m1, m2], dtype=np.float32)



# revision 4
# speedup vs baseline: 154.6393x; 154.6393x over previous
"""Co-teaching loss (drop-region CE) kernel for Trainium2, 8 NeuronCores.

Reference computation:
  - 2x2 maxpool on inputs1/inputs2 [8,19,512,512] and targets [8,512,512]
  - per-pixel CE loss of each pooled input vs pooled targets -> [8, 65536] x2
  - per-row ascending argsort of each loss map, keep num_remember smallest,
    gather the *other* loss at those indices, return the two scalar means.

Distribution: data-parallel over batch B=8, one batch row per NeuronCore.
Each core computes its row's two pooled CE loss maps on-device (that is all
of the memory-bound work: ~41MB of f32 input reads per core).  The tiny
top-k selection over the [8, 65536] loss maps is done on host exactly like
the reference (stable argsort semantics).

On-device pipeline per core (v2 — HWDGE streaming):
  - one 1MB HWDGE DMA per channel (f32, no cast): tile [128, 2048] where
    partition p holds input rows 4p..4p+3.  All input loads ride the sync
    (SP) HWDGE ring back-to-back; targets + stores use the scalar ring.
  - h-pool on DVE (f32 in -> bf16 out) with an interleaved output layout
    [ro, wi, j] so the w-pool is a contiguous step-1 bf16 TT max (2x mode).
    Pooled pixel (2p+ro, j) lives at partition p, offset ro*256+j.
  - masks (tp == c) on GPSIMD; x_t gather via copy_predicated (DVE, bf16);
    exp in place on ACT; channel sum as an in-place TT add tree (5 ops);
    Ln on ACT; loss = lnS - x_t (f32) -> DMA out, already in row-major
    pooled-pixel order.
"""

import numpy as np

B, C, H, W = 8, 19, 512, 512
HP, WP = 256, 256  # pooled spatial dims
L = HP * WP
N_CORES = 8
PIX = 512          # pooled pixels per partition (rows 2p, 2p+1)

_prog_cache = {}


def _build_program(repeat=1):
    from contextlib import ExitStack

    import concourse.bass as bass  # noqa: F401
    import concourse.mybir as mybir
    import concourse.tile as tile
    from concourse import bacc

    f32 = mybir.dt.float32
    bf16 = mybir.dt.bfloat16
    i32 = mybir.dt.int32
    u8 = mybir.dt.uint8
    Alu = mybir.AluOpType
    Act = mybir.ActivationFunctionType

    nc = bacc.Bacc("TRN2", target_bir_lowering=False, debug=False,
                   num_devices=N_CORES)

    x_in = [
        nc.dram_tensor("x1", [C, H, W], f32, kind="ExternalInput"),
        nc.dram_tensor("x2", [C, H, W], f32, kind="ExternalInput"),
    ]
    tg = nc.dram_tensor("tg", [H, W], i32, kind="ExternalInput")
    l_out = [
        nc.dram_tensor("loss1", [HP, WP], f32, kind="ExternalOutput"),
        nc.dram_tensor("loss2", [HP, WP], f32, kind="ExternalOutput"),
    ]

    def chan_view(t2d):
        # [512, 512] DRAM -> [128, 2048]; partition p = rows 4p..4p+3
        return t2d.rearrange("h w -> (h w)").rearrange("(p n) -> p n", p=128)

    with tile.TileContext(nc) as tc, ExitStack() as ctx:
        raw_pool = ctx.enter_context(tc.tile_pool(name="raw", bufs=6))
        h_pool = ctx.enter_context(tc.tile_pool(name="hp", bufs=3))
        p_pool = ctx.enter_context(tc.tile_pool(name="pooled", bufs=1))
        tgt_pool = ctx.enter_context(tc.tile_pool(name="tgt", bufs=1))
        mask_pool = ctx.enter_context(tc.tile_pool(name="mask", bufs=1))
        small = ctx.enter_context(tc.tile_pool(name="small", bufs=1))

        for rep in range(repeat):
            if rep:
                tc.strict_bb_all_engine_barrier()

            # ---- pooled targets: tp [128, 512] i32 (pixel = ro*256+j) ----
            traw = tgt_pool.tile([128, 2048], i32, tag="traw")
            nc.scalar.dma_start(out=traw[:], in_=chan_view(tg))
            tv = traw[:].rearrange("p (r j wi) -> p r j wi", r=4, wi=2)
            th = tgt_pool.tile([128, 1024], i32, tag="th")
            th_w = th[:].rearrange("p (ro wi j) -> p ro j wi", ro=2, wi=2)
            nc.vector.tensor_tensor(out=th_w, in0=tv[:, 0:4:2], in1=tv[:, 1:4:2],
                                    op=Alu.max)
            th_v = th[:].rearrange("p (ro wi j) -> p wi ro j", ro=2, wi=2)
            tp = tgt_pool.tile([128, PIX], i32, tag="tp")
            tp_v = tp[:].rearrange("p (ro j) -> p ro j", ro=2)
            nc.vector.tensor_tensor(out=tp_v, in0=th_v[:, 0], in1=th_v[:, 1],
                                    op=Alu.max)

            # ---- masks (tp == c) for c = 1..18, shared by both inputs ----
            masks = mask_pool.tile([128, 18 * PIX], u8, tag="masks")
            for c in range(1, C):
                nc.gpsimd.tensor_scalar(
                    out=masks[:, (c - 1) * PIX:c * PIX], in0=tp[:],
                    scalar1=float(c), scalar2=None, op0=Alu.is_equal)

            # ---- per input: stream channels, pool, gather x_t, exp ----
            P_a = p_pool.tile([128, C * PIX], bf16, tag="P0")
            P_b = p_pool.tile([128, C * PIX], bf16, tag="P1")
            P12 = [P_a, P_b]
            xt12 = small.tile([128, 2 * PIX], bf16, tag="xt")
            S12 = small.tile([128, 2 * PIX], f32, tag="S")
            lt12 = small.tile([128, 2 * PIX], f32, tag="loss")
            for xi in range(2):
                P = P12[xi]
                xt = xt12[:, xi * PIX:(xi + 1) * PIX]
                for c in range(C):
                    T = raw_pool.tile([128, 2048], f32, tag="T")
                    nc.sync.dma_start(out=T[:], in_=chan_view(x_in[xi][c]))
                    Tv = T[:].rearrange("p (r j wi) -> p r j wi", r=4, wi=2)
                    Hc = h_pool.tile([128, 1024], bf16, tag="H")
                    Hw = Hc[:].rearrange("p (ro wi j) -> p ro j wi",
                                         ro=2, wi=2)
                    nc.vector.tensor_tensor(out=Hw, in0=Tv[:, 0:4:2],
                                            in1=Tv[:, 1:4:2], op=Alu.max)
                    Hv = Hc[:].rearrange("p (ro wi j) -> p wi ro j",
                                         ro=2, wi=2)
                    Pc = P[:, c * PIX:(c + 1) * PIX]
                    Pc_v = Pc.rearrange("p (ro j) -> p ro j", ro=2)
                    nc.vector.tensor_tensor(out=Pc_v, in0=Hv[:, 0],
                                            in1=Hv[:, 1], op=Alu.max)
                    # x_t update for this channel, then exp in place
                    if c == 0:
                        nc.vector.tensor_copy(xt, Pc)
                    else:
                        nc.vector.copy_predicated(
                            out=xt, mask=masks[:, (c - 1) * PIX:c * PIX],
                            data=Pc)
                    nc.scalar.activation(out=Pc, in_=Pc, func=Act.Exp)

                # ---- channel sum: in-place TT add tree over exp'd P ----
                nc.vector.tensor_tensor(
                    out=P[:, 0:3 * PIX], in0=P[:, 0:3 * PIX],
                    in1=P[:, 16 * PIX:19 * PIX], op=Alu.add)
                nc.vector.tensor_tensor(
                    out=P[:, 0:8 * PIX], in0=P[:, 0:8 * PIX],
                    in1=P[:, 8 * PIX:16 * PIX], op=Alu.add)
                nc.vector.tensor_tensor(
                    out=P[:, 0:4 * PIX], in0=P[:, 0:4 * PIX],
                    in1=P[:, 4 * PIX:8 * PIX], op=Alu.add)
                nc.vector.tensor_tensor(
                    out=P[:, 0:2 * PIX], in0=P[:, 0:2 * PIX],
                    in1=P[:, 2 * PIX:4 * PIX], op=Alu.add)
                S = S12[:, xi * PIX:(xi + 1) * PIX]
                nc.vector.tensor_tensor(
                    out=S, in0=P[:, 0:PIX], in1=P[:, PIX:2 * PIX],
                    op=Alu.add)
                lnS = lt12[:, xi * PIX:(xi + 1) * PIX]
                nc.scalar.activation(out=lnS, in_=S, func=Act.Ln)
                nc.vector.tensor_sub(lnS, lnS, xt)
                nc.scalar.dma_start(out=chan_view_out(l_out[xi]), in_=lnS)

    nc.compile()
    return nc


def chan_view_out(t2d):
    # [256, 256] DRAM -> [128, 512]; partition p = pooled rows 2p, 2p+1
    return t2d.rearrange("h w -> (h w)").rearrange("(p n) -> p n", p=128)


def _get_program():
    if "nc" not in _prog_cache:
        _prog_cache["nc"] = _build_program()
    return _prog_cache["nc"]


def _device_loss_maps(inputs1, inputs2, targets):
    """Run the 8-core SPMD kernel; return loss1, loss2 as [8, 65536] f32."""
    from concourse.bass_utils import run_bass_kernel_spmd

    nc = _get_program()
    in_maps = [
        {
            "x1": np.ascontiguousarray(inputs1[b], dtype=np.float32),
            "x2": np.ascontiguousarray(inputs2[b], dtype=np.float32),
            "tg": np.ascontiguousarray(targets[b], dtype=np.int32),
        }
        for b in range(B)
    ]
    res = run_bass_kernel_spmd(nc, in_maps, list(range(N_CORES)))
    loss1 = np.stack([np.asarray(res.results[b]["loss1"]).reshape(L)
                      for b in range(B)])
    loss2 = np.stack([np.asarray(res.results[b]["loss2"]).reshape(L)
                      for b in range(B)])
    return loss1, loss2


def kernel(inputs1, inputs2, targets, forget_rate):
    inputs1 = np.asarray(inputs1, dtype=np.float32)
    inputs2 = np.asarray(inputs2, dtype=np.float32)
    targets = np.asarray(targets, dtype=np.int32)

    loss1, loss2 = _device_loss_maps(inputs1, inputs2, targets)

    num_remember = int((1.0 - float(forget_rate)) * L)
    # stable ascending argsort (matches jnp.argsort) -> keep smallest k,
    # gather the swapped loss, mean.
    ind1 = np.argsort(loss1, axis=1, kind="stable")[:, :num_remember]
    ind2 = np.argsort(loss2, axis=1, kind="stable")[:, :num_remember]
    m1 = np.take_along_axis(loss1, ind2, axis=1).mean(dtype=np.float64)
    m2 = np.take_along_axis(loss2, ind1, axis=1).mean(dtype=np.float64)
    return np.array([m1, m2], dtype=np.float32)
